# revision 4
# baseline (speedup 1.0000x reference)
"""Trainium2 Bass kernel for broadcast subtract (vq codebook diff).

Computes diff[k, n, d] = input_x[n, d] - input_centroid[k, d]
  input_x:        [65536, 64] f32
  input_centroid: [32, 64]    f32
  output:         [32, 65536, 64] f32   (512 MiB)

Sharding: data-parallel along N across 8 cores (8192 points per core);
centroid table replicated. Per-core traffic: 2 MiB read + 64 MiB write
-> memory (HBM write) bound.

Per-core layout: x rows on the 128 SBUF partitions. With
n_local = a*4096 + p*32 + b (p = partition, b packed with d into the
free dim), every [128, 2048] f32 tile is a fully contiguous 1 MiB
region both for the x loads and for each out[k] store, so every DMA
moves 1 MiB with 8 KiB per partition line. DVE does the broadcast
subtract (centroid replicated across partitions once at startup).
"""

import numpy as np

N = 65536
K = 32
D = 64
NCORES = 8
NLOC = N // NCORES  # 8192 rows per core
P = 128             # SBUF partitions
B = 32              # n-rows packed into the free dim per partition
A = NLOC // (P * B)  # 2 outer tiles per core

_COMPILED = {}


def _build_bass():
    import concourse.bacc as bacc
    import concourse.mybir as mybir
    from concourse import tile

    f32 = mybir.dt.float32

    nc = bacc.Bacc(None)
    x = nc.dram_tensor("x", [NLOC, D], f32, kind="ExternalInput")
    cent = nc.dram_tensor("cent", [K, D], f32, kind="ExternalInput")
    out = nc.dram_tensor("out", [K, NLOC, D], f32, kind="ExternalOutput")

    # [A, P, B*D] view; per (a) tile is 1 MiB contiguous in DRAM.
    x_r = x.rearrange("(a p b) d -> a p (b d)", p=P, b=B)
    # per (k, a): same geometry inside out[k].
    out_r = out.rearrange("k (a p b) d -> k a p (b d)", p=P, b=B)

    with tile.TileContext(nc) as tc:
        with (
            tc.tile_pool(name="cent_pool", bufs=1) as cent_pool,
            tc.tile_pool(name="x_pool", bufs=2) as x_pool,
            tc.tile_pool(name="o_pool", bufs=6) as o_pool,
        ):
            # Replicate the [K, D] centroid table across all 128 partitions
            # with a partition-broadcast DMA (source step 0 over partitions).
            cent_sb = cent_pool.tile([P, K * D], f32)
            cent_flat = cent.rearrange("k d -> (k d)")
            nc.sync.dma_start(
                out=cent_sb[:], in_=cent_flat[None, :].broadcast_to([P, K * D])
            )

            for a in range(A):
                x_t = x_pool.tile([P, B * D], f32, tag="x")
                nc.sync.dma_start(out=x_t[:], in_=x_r[a])
                x3 = x_t.rearrange("p (b d) -> p b d", d=D)
                for k in range(K):
                    o_t = o_pool.tile([P, B * D], f32, tag="o")
                    c_k = cent_sb[:, None, k * D:(k + 1) * D].broadcast_to([P, B, D])
                    nc.vector.tensor_sub(
                        o_t.rearrange("p (b d) -> p b d", d=D), x3, c_k
                    )
                    nc.sync.dma_start(out=out_r[k, a], in_=o_t[:])

    nc.finalize()
    return nc


def _get_nc():
    if "nc" not in _COMPILED:
        _COMPILED["nc"] = _build_bass()
    return _COMPILED["nc"]


def run_sharded(input_x: np.ndarray, input_centroid: np.ndarray, trace: bool = False):
    """Shard, run on 8 cores, gather. Returns (full_output, BassKernelResults)."""
    from concourse.bass_utils import run_bass_kernel_spmd

    x = np.ascontiguousarray(np.asarray(input_x, dtype=np.float32))
    c = np.ascontiguousarray(np.asarray(input_centroid, dtype=np.float32))
    assert x.shape == (N, D) and c.shape == (K, D)

    nc = _get_nc()
    in_maps = [
        {"x": x[i * NLOC:(i + 1) * NLOC], "cent": c} for i in range(NCORES)
    ]
    res = run_bass_kernel_spmd(nc, in_maps, core_ids=list(range(NCORES)), trace=trace)
    full = np.concatenate([r["out"] for r in res.results], axis=1)
    return full, res


def kernel(input_x: np.ndarray, input_centroid: np.ndarray) -> np.ndarray:
    full, _ = run_sharded(input_x, input_centroid, trace=False)
    return full
